# revision 58
# baseline (speedup 1.0000x reference)
"""Trainium2 Bass kernel for a 2-block decoder transformer.

Model (hardcoded): B=2, S=2048, D=512, V=32000, H=11.
  emb = table[x] + pos;  two attention blocks (softmax over the QUERY axis,
  causal mask); logits = res @ out_w + out_b.

Sharding: the [512,32000] vocab projection is column-split 8 ways (4000
cols/core); the (cheap) attention stack is replicated on every core, so no
collectives are needed.  Each core returns its [4096,4000] logit slice and the
host concatenates.

All matmuls run in float32r (fp32-in-memory, reduced-precision single-pass PE
mode, 1 cycle/row) with fp32 PSUM accumulation.
"""

import numpy as np

try:  # persistent compile cache (saves ~5 min on repeat runs in one machine)
    import jax
    jax.config.update("jax_compilation_cache_dir", "/tmp/jax_kernel_cache")
    jax.config.update("jax_persistent_cache_min_entry_size_bytes", -1)
    jax.config.update("jax_persistent_cache_min_compile_time_secs", 0)
except Exception:
    pass

import concourse.bass as bass
import concourse.tile as tile
from concourse import bacc, mybir
from concourse.bass_utils import run_bass_kernel_spmd
from concourse.masks import make_identity

B, S, D, V, H = 2, 2048, 512, 32000, 11
HP = 12                      # H padded even (f32r matmul: even moving free dim)
NCORES = 8
TOK = B * S                  # 4096 tokens
DT = D // 128                # 4 d-tiles
VS = V // NCORES             # 4000 vocab cols per core
SCH = 512                    # sequence chunk (matmul free dim)
NSCH_B = S // SCH            # 4 chunks per batch
KT_B = S // 128              # 16 k-tiles per batch
NQUART = 4                   # vocab quarters per core
VQ = VS // NQUART            # 1000
VCH = 500                    # vocab chunk (one PSUM bank)
NEG = -1e30

f32 = mybir.dt.float32
f32r = mybir.dt.float32r
i16 = mybir.dt.int16

_cached = {}


def _pos_matrix_np():
    pos = np.arange(S, dtype=np.float32)[:, None]
    i = np.arange(0, D, 2, dtype=np.float32)
    denom = np.power(np.float32(10000.0), 2.0 * i / D).astype(np.float32)
    ang = pos / denom[None, :]
    store = np.zeros((S, D), dtype=np.float32)
    store[:, 0::2] = np.sin(ang)
    store[:, 1::2] = np.cos(ang)
    return store


def _masks_np():
    # mask[r, p, qf] = 0 if qf >= r*128 + p else NEG   (boundary-tile causal mask)
    qf = np.arange(SCH)[None, None, :]
    p = np.arange(128)[None, :, None]
    r = np.arange(4)[:, None, None]
    return np.where(qf >= r * 128 + p, 0.0, NEG).astype(np.float32)


def _rearr_w(w):
    # [D, F] -> [128, DT, F]  (d = dt*128 + p)
    return np.ascontiguousarray(w.reshape(DT, 128, -1).transpose(1, 0, 2))


def _attn_block(nc, tc, act_in, act_out, wk, wq, wv, fw, kb, qb, vbb, fb,
                maskt, kqv, exppool, smalls, ps, blk, interleave=None):
    """One attention block: act_in [128, DT, TOK] f32r -> act_out (same shape)."""
    kT = kqv.tile([11, TOK], f32r, tag="kT", name=f"kT{blk}")
    qT = kqv.tile([11, TOK], f32r, tag="qT", name=f"qT{blk}")
    vnat = kqv.tile([128, TOK // 128, HP], f32r, tag="vnat", name=f"vnat{blk}")
    resT = kqv.tile([11, TOK], f32r, tag="resT", name=f"resT{blk}")

    # --- projections k,q: [11, TOK] = w.T @ eT, + bias (bias add on ScalarE) ---
    for ch in range(TOK // SCH):
        sl = slice(ch * SCH, (ch + 1) * SCH)
        for w_sb, b_sb, dest in ((wk, kb, kT), (wq, qb, qT)):
            pst = ps.tile([11, SCH], f32, tag="bank1", bufs=4,
                          name=f"proj{blk}_{ch}")
            for d in range(DT):
                nc.tensor.matmul(pst[:], w_sb[:, d], act_in[:, d, sl],
                                 start=(d == 0), stop=(d == DT - 1))
            nc.scalar.add(dest[:, sl], pst[:], b_sb[:])
    # --- v directly in [tok, HP] layout: eT_tile.T @ wv  (HP=12: f32r matmul
    # needs an even moving free dim, so wv is zero-padded to 12 cols) ---
    for t in range(TOK // 128):
        tsl = slice(t * 128, (t + 1) * 128)
        pst = ps.tile([128, HP], f32, tag="bank1", bufs=4, name=f"vproj{blk}_{t}")
        for d in range(DT):
            nc.tensor.matmul(pst[:], act_in[:, d, tsl], wv[:, d],
                             start=(d == 0), stop=(d == DT - 1))
        nc.vector.tensor_add(vnat[:, t], pst[:], vbb[:])

    # --- attention per batch ---
    for b in range(B):
        base = b * S
        avps = [ps.tile([11, SCH], f32, tag="bank1", bufs=4,
                        name=f"av{blk}_{b}_{c}") for c in range(NSCH_B)]
        for t in range(KT_B):
            c0 = t // 4
            ksl = slice(base + t * 128, base + t * 128 + 128)
            # chunk pairs: (start_chunk, n_chunks) covering c0..3
            pairs = {0: [(0, 2), (2, 2)], 1: [(1, 1), (2, 2)],
                     2: [(2, 2)], 3: [(3, 1)]}[c0]
            exp_t = exppool.tile([128, NSCH_B, SCH], f32r, tag="exp",
                                 name=f"exp{blk}_{b}_{t}")
            sums = smalls.tile([128, 2], f32, tag="sums", name=f"sums{blk}_{b}_{t}")
            for pi, (cs, w) in enumerate(pairs):
                pst = ps.tile([128, 2, SCH], f32, tag="pair", bufs=2,
                              name=f"sc{blk}_{b}_{t}_{pi}")
                for j in range(w):
                    qsl = slice(base + (cs + j) * SCH, base + (cs + j + 1) * SCH)
                    nc.tensor.matmul(pst[:, j], kT[:, ksl], qT[:, qsl],
                                     start=True, stop=True)
                if pi == 0:  # boundary chunk c0 is the first chunk of pair 0
                    nc.vector.tensor_add(pst[:, 0], pst[:, 0], maskt[:, t % 4])
                nc.scalar.activation(exp_t[:, cs:cs + w], pst[:, :w],
                                     mybir.ActivationFunctionType.Exp,
                                     accum_out=sums[:, pi:pi + 1])
            if len(pairs) > 1:
                stot = smalls.tile([128, 1], f32, tag="stot", name=f"st{blk}_{b}_{t}")
                nc.vector.reduce_sum(stot[:], sums[:, :2], axis=mybir.AxisListType.X)
            else:
                stot = sums[:, 0:1]
            rec = smalls.tile([128, 1], f32, tag="rec", name=f"rec{blk}_{b}_{t}")
            nc.vector.reciprocal(rec[:], stot[:])
            vsc = smalls.tile([128, HP], f32r, tag="vsc", name=f"vsc{blk}_{b}_{t}")
            nc.vector.tensor_scalar_mul(vsc[:], vnat[:, b * KT_B + t], rec[:])
            for c in range(c0, NSCH_B):
                nc.tensor.matmul(avps[c][:], vsc[:, :H], exp_t[:, c],
                                 start=(t == 0), stop=(t == 4 * c + 3))
                if t == 4 * c + 3:
                    nc.vector.tensor_copy(
                        resT[:, base + c * SCH:base + (c + 1) * SCH], avps[c][:])
            if b == 1 and interleave is not None:
                interleave(t)

        # --- ff for this batch: act_out[d, tok] = fw.T @ resT + fb ---
        for ch in range(b * NSCH_B, (b + 1) * NSCH_B):
            sl = slice(ch * SCH, (ch + 1) * SCH)
            for d in range(DT):
                pst = ps.tile([128, SCH], f32, tag="bank1", bufs=4,
                              name=f"ff{blk}_{d}_{ch}")
                nc.tensor.matmul(pst[:], fw[:, d * 128:(d + 1) * 128],
                                 resT[:, sl], start=True, stop=True)
                if ch % 2 == 0:
                    nc.vector.tensor_scalar_add(act_out[:, d, sl], pst[:],
                                                fb[:, d:d + 1])
                else:
                    nc.scalar.add(act_out[:, d, sl], pst[:], fb[:, d:d + 1])


def build():
    nc = bacc.Bacc("TRN2", target_bir_lowering=False, debug=False,
                   num_devices=NCORES)
    d_emb = nc.dram_tensor("emb_table", [V, D], f32, kind="ExternalInput").ap()
    d_idx = nc.dram_tensor("idx", [128, TOK // 16], i16, kind="ExternalInput").ap()
    d_pos = nc.dram_tensor("pos_t", [128, DT, S], f32, kind="ExternalInput").ap()
    d_masks = nc.dram_tensor("masks", [128, 4, SCH], f32, kind="ExternalInput").ap()
    d_wkqv = []
    d_bias = []
    for blk in range(2):
        d_wkqv.append([
            nc.dram_tensor(f"w{blk}_k", [128, DT, H], f32, kind="ExternalInput").ap(),
            nc.dram_tensor(f"w{blk}_q", [128, DT, H], f32, kind="ExternalInput").ap(),
            nc.dram_tensor(f"w{blk}_v", [128, DT, HP], f32, kind="ExternalInput").ap()])
        d_wkqv[blk].append(
            nc.dram_tensor(f"w{blk}_f", [H, D], f32, kind="ExternalInput").ap())
        d_bias.append([
            nc.dram_tensor(f"b{blk}_k", [11, 1], f32, kind="ExternalInput").ap(),
            nc.dram_tensor(f"b{blk}_q", [11, 1], f32, kind="ExternalInput").ap(),
            nc.dram_tensor(f"b{blk}_v", [128, HP], f32, kind="ExternalInput").ap()])
        d_bias[blk].append(
            nc.dram_tensor(f"b{blk}_f", [128, DT], f32, kind="ExternalInput").ap())
    d_wout = nc.dram_tensor("w_out", [128, DT, VS], f32, kind="ExternalInput").ap()
    d_bout = nc.dram_tensor("b_out", [128, VS], f32, kind="ExternalInput").ap()
    d_out = nc.dram_tensor("out", [TOK, VS], f32, kind="ExternalOutput").ap()

    with tile.TileContext(nc) as tc:
        import contextlib
        with contextlib.ExitStack() as ctx:
            consts = ctx.enter_context(tc.tile_pool(name="consts", bufs=1))
            acts = ctx.enter_context(tc.tile_pool(name="acts", bufs=1))
            ps = ctx.enter_context(tc.tile_pool(name="ps", bufs=1, space="PSUM"))

            ident = consts.tile([128, 128], f32, tag="ident", name="ident")
            make_identity(nc, ident[:])
            idxt = consts.tile([128, TOK // 16], i16, tag="idx", name="idxt")
            nc.sync.dma_start(idxt[:], d_idx)
            maskt = consts.tile([128, 4, SCH], f32, tag="masks", name="maskt")

            # attention weights/biases (small, resident)
            wb = []
            for blk in range(2):
                wk = consts.tile([128, DT, H], f32r, tag=f"w{blk}k", name=f"w{blk}k")
                wq = consts.tile([128, DT, H], f32r, tag=f"w{blk}q", name=f"w{blk}q")
                wv = consts.tile([128, DT, HP], f32r, tag=f"w{blk}v", name=f"w{blk}v")
                fw = consts.tile([11, D], f32r, tag=f"w{blk}f", name=f"w{blk}f")
                kb = consts.tile([11, 1], f32, tag=f"b{blk}k", name=f"b{blk}k")
                qb = consts.tile([11, 1], f32, tag=f"b{blk}q", name=f"b{blk}q")
                vbb = consts.tile([128, HP], f32, tag=f"b{blk}v", name=f"b{blk}v")
                fb = consts.tile([128, DT], f32, tag=f"b{blk}f", name=f"b{blk}f")
                for t_sb, t_dr in zip((kb, qb, vbb, fb), d_bias[blk]):
                    nc.sync.dma_start(t_sb[:], t_dr)
                wb.append((wk, wq, wv, fw, kb, qb, vbb, fb))

            # vocab-projection weight streaming pool opened early so the first
            # quarters' DMAs can prefetch during attention
            wpool = ctx.enter_context(tc.tile_pool(name="wpool", bufs=2))
            stpool = ctx.enter_context(tc.tile_pool(name="stpool", bufs=3))
            obpool = ctx.enter_context(tc.tile_pool(name="obpool", bufs=2))
            wq_tiles = []

            eT = acts.tile([128, DT, TOK], f32r, tag="act", name="eT")

            # --- embedding gather + transpose + positional encoding ---
            with (tc.tile_pool(name="embed", bufs=2) as epool,
                  tc.tile_pool(name="pospool", bufs=1) as ppool):
                post = ppool.tile([128, DT, S], f32, tag="pos", name="post")
                for g in range(TOK // 512):
                    if g < 4:  # batch-0 pass loads pos just-in-time; batch 1 reuses
                        nc.sync.dma_start(post[:, :, g * 512:(g + 1) * 512],
                                          d_pos[:, :, g * 512:(g + 1) * 512])
                    en = epool.tile([128, 4, D], f32, tag="enat", name=f"en{g}")
                    nc.gpsimd.dma_gather(
                        out_ap=en[:], in_ap=d_emb,
                        idxs_ap=idxt[:, 32 * g:32 * (g + 1)],
                        num_idxs=512, num_idxs_reg=512, elem_size=D)
                    for c4 in range(4):
                        tokbase = g * 512 + c4 * 128
                        sbase = tokbase % S
                        for d in range(DT):
                            pst = ps.tile([128, 128], f32, tag="bank1", bufs=4,
                                          name=f"etr{g}_{c4}_{d}")
                            nc.tensor.transpose(
                                pst[:], en[:, c4, d * 128:(d + 1) * 128], ident[:])
                            nc.vector.tensor_add(
                                eT[:, d, tokbase:tokbase + 128], pst[:],
                                post[:, d, sbase:sbase + 128])

            # attention-weight cast-DMAs emitted after the gathers so they
            # don't delay them on the single SWDGE queue
            for blk in range(2):
                for t_sb, t_dr in zip(wb[blk][:4], d_wkqv[blk]):
                    nc.gpsimd.dma_start(t_sb[:], t_dr)
            # prefetch first W quarter during attention; per-d pieces so the
            # vocab matmuls can start on partial arrival
            nc.sync.dma_start(maskt[:], d_masks)

            def load_wq(q):
                wq_sb = wpool.tile([128, DT, VQ], f32r, tag="W", name=f"W{q}")
                for d in range(DT):
                    nc.gpsimd.dma_start(wq_sb[:, d],
                                        d_wout[:, d, q * VQ:(q + 1) * VQ])
                return wq_sb

            wq_tiles.append(load_wq(0))
            ob_q0 = obpool.tile([128, VQ], f32, tag="ob", name="ob0")
            nc.sync.dma_start(ob_q0[:], d_bout[:, 0:VQ])

            # --- two attention blocks ---
            with (tc.tile_pool(name="kqv", bufs=1) as kqv,
                  tc.tile_pool(name="exppool", bufs=3) as exppool,
                  tc.tile_pool(name="smalls", bufs=6) as smalls):
                res1 = acts.tile([128, DT, TOK], f32r, tag="act", name="res1")
                _attn_block(nc, tc, eT, res1, *wb[0], maskt,
                            kqv, exppool, smalls, ps, 0)
                res2 = acts.tile([128, DT, TOK], f32r, tag="act", name="res2")

                def bigmm_unit(q, t, wq_sb, ob_q):
                    tsl = slice(t * 128, (t + 1) * 128)
                    stage = stpool.tile([128, VQ], f32, tag="stage",
                                        name=f"stg{q}_{t}")
                    for c in range(VQ // VCH):
                        pst = ps.tile([128, VCH], f32, tag="bank1", bufs=4,
                                      name=f"big{q}_{t}_{c}",
                                      padded_shape=[128, 512])
                        csl = slice(c * VCH, (c + 1) * VCH)
                        for d in range(DT):
                            nc.tensor.matmul(pst[:], res2[:, d, tsl],
                                             wq_sb[:, d, csl],
                                             start=(d == 0), stop=(d == DT - 1))
                        nc.vector.tensor_add(stage[:, csl], pst[:], ob_q[:, csl])
                    nc.sync.dma_start(d_out[tsl, q * VQ:(q + 1) * VQ], stage[:])

                # vocab-matmul units for quarter 0 / batch-0 tok-tiles 4..15
                # interleave into block-2's batch-1 loop, using the bank1
                # PSUM slots that the AV accumulators release progressively
                # (av[c] frees at k-tile 4c+3); the score-pair pipeline is
                # untouched, so ACT keeps streaming exp at full rate.
                _attn_block(nc, tc, res1, res2, *wb[1], maskt,
                            kqv, exppool, smalls, ps, 1,
                            interleave=lambda t: (bigmm_unit(
                                0, t, wq_tiles[0], ob_q0) if t >= 4 else None))

            # --- vocab projection: out[tok, v] = res2.T @ W + b ---
            if True:
                for q in range(NQUART):
                    wq_sb = wq_tiles[q] if q < len(wq_tiles) else load_wq(q)
                    if q == 0:
                        ob_q = ob_q0
                    else:
                        ob_q = obpool.tile([128, VQ], f32, tag="ob",
                                           name=f"ob{q}")
                        nc.sync.dma_start(ob_q[:], d_bout[:, q * VQ:(q + 1) * VQ])
                    for t in range(TOK // 128):
                        if q == 0 and 4 <= t < KT_B:
                            continue  # emitted interleaved with attention
                        bigmm_unit(q, t, wq_sb, ob_q)

    nc.compile()
    return nc


def _prep_inputs(inputs):
    x = np.asarray(inputs["x"]).reshape(-1).astype(np.int64)
    assert x.size == TOK
    idx16 = x.astype(np.int16)
    idx_w = np.tile(np.ascontiguousarray(idx16.reshape(TOK // 16, 16).T), (8, 1))

    pos_t = _rearr_w(np.ascontiguousarray(_pos_matrix_np().T))  # [128, DT, S]
    masks = np.ascontiguousarray(_masks_np().transpose(1, 0, 2))  # [128, 4, SCH]

    common = {
        "emb_table": np.ascontiguousarray(np.asarray(inputs["emb_table"], dtype=np.float32)),
        "idx": np.ascontiguousarray(idx_w),
        "pos_t": np.ascontiguousarray(pos_t),
        "masks": masks,
    }
    for blk, pre in ((0, "1"), (1, "2")):
        for n, key in (("k", f"k{pre}_w"), ("q", f"q{pre}_w")):
            common[f"w{blk}_{n}"] = _rearr_w(np.asarray(inputs[key], dtype=np.float32))
        wv_pad = np.zeros((D, HP), dtype=np.float32)
        wv_pad[:, :H] = np.asarray(inputs[f"v{pre}_w"], dtype=np.float32)
        common[f"w{blk}_v"] = _rearr_w(wv_pad)
        common[f"w{blk}_f"] = np.ascontiguousarray(
            np.asarray(inputs[f"ff{pre}_w"], dtype=np.float32))
        for n, key in (("k", f"k{pre}_b"), ("q", f"q{pre}_b")):
            common[f"b{blk}_{n}"] = np.ascontiguousarray(
                np.asarray(inputs[key], dtype=np.float32).reshape(11, 1))
        vb_pad = np.zeros(HP, dtype=np.float32)
        vb_pad[:H] = np.asarray(inputs[f"v{pre}_b"], dtype=np.float32)
        common[f"b{blk}_v"] = np.ascontiguousarray(
            np.broadcast_to(vb_pad, (128, HP)))
        common[f"b{blk}_f"] = np.ascontiguousarray(
            np.asarray(inputs[f"ff{pre}_b"], dtype=np.float32).reshape(DT, 128).T)

    out_w = np.asarray(inputs["out_w"], dtype=np.float32)
    out_b = np.asarray(inputs["out_b"], dtype=np.float32)
    in_maps = []
    for c in range(NCORES):
        m = dict(common)
        m["w_out"] = _rearr_w(np.ascontiguousarray(out_w[:, c * VS:(c + 1) * VS]))
        m["b_out"] = np.ascontiguousarray(
            np.broadcast_to(out_b[c * VS:(c + 1) * VS], (128, VS)))
        in_maps.append(m)
    return in_maps


_last_results = None


def kernel(**inputs) -> np.ndarray:
    global _last_results
    if "nc" not in _cached:
        _cached["nc"] = build()
    nc = _cached["nc"]
    in_maps = _prep_inputs(inputs)
    res = run_bass_kernel_spmd(nc, in_maps, core_ids=list(range(NCORES)))
    _last_results = res
    out = np.concatenate([res.results[c]["out"] for c in range(NCORES)], axis=1)
    return out.reshape(B, S, V)


# revision 65
# speedup vs baseline: 1.0328x; 1.0328x over previous
"""Trainium2 Bass kernel for a 2-block decoder transformer.

Model (hardcoded): B=2, S=2048, D=512, V=32000, H=11.
  emb = table[x] + pos;  two attention blocks (softmax over the QUERY axis,
  causal mask); logits = res @ out_w + out_b.

Sharding: the [512,32000] vocab projection is column-split 8 ways (4000
cols/core); the (cheap) attention stack is replicated on every core, so no
collectives are needed.  Each core returns its [4096,4000] logit slice and the
host concatenates.

All matmuls run in float32r (fp32-in-memory, reduced-precision single-pass PE
mode, 1 cycle/row) with fp32 PSUM accumulation.
"""

import numpy as np

try:  # persistent compile cache (saves ~5 min on repeat runs in one machine)
    import jax
    jax.config.update("jax_compilation_cache_dir", "/tmp/jax_kernel_cache")
    jax.config.update("jax_persistent_cache_min_entry_size_bytes", -1)
    jax.config.update("jax_persistent_cache_min_compile_time_secs", 0)
except Exception:
    pass

import concourse.bass as bass
import concourse.tile as tile
from concourse import bacc, mybir
from concourse.bass_utils import run_bass_kernel_spmd
from concourse.masks import make_identity

B, S, D, V, H = 2, 2048, 512, 32000, 11
HP = 12                      # H padded even (f32r matmul: even moving free dim)
NCORES = 8
TOK = B * S                  # 4096 tokens
DT = D // 128                # 4 d-tiles
VS = V // NCORES             # 4000 vocab cols per core
SCH = 512                    # sequence chunk (matmul free dim)
NSCH_B = S // SCH            # 4 chunks per batch
KT_B = S // 128              # 16 k-tiles per batch
NQUART = 4                   # vocab quarters per core
VQ = VS // NQUART            # 1000
VCH = 500                    # vocab chunk (one PSUM bank)
NEG = -1e30

f32 = mybir.dt.float32
f32r = mybir.dt.float32r
i16 = mybir.dt.int16

_cached = {}


def _pos_matrix_np():
    pos = np.arange(S, dtype=np.float32)[:, None]
    i = np.arange(0, D, 2, dtype=np.float32)
    denom = np.power(np.float32(10000.0), 2.0 * i / D).astype(np.float32)
    ang = pos / denom[None, :]
    store = np.zeros((S, D), dtype=np.float32)
    store[:, 0::2] = np.sin(ang)
    store[:, 1::2] = np.cos(ang)
    return store


def _masks_np():
    # mask[r, p, qf] = 0 if qf >= r*128 + p else NEG   (boundary-tile causal mask)
    qf = np.arange(SCH)[None, None, :]
    p = np.arange(128)[None, :, None]
    r = np.arange(4)[:, None, None]
    return np.where(qf >= r * 128 + p, 0.0, NEG).astype(np.float32)


def _rearr_w(w):
    # [D, F] -> [128, DT, F]  (d = dt*128 + p)
    return np.ascontiguousarray(w.reshape(DT, 128, -1).transpose(1, 0, 2))


def _attn_block(nc, tc, act_in, act_out, wk, wq, wv, fw, kb, qb, vbb, fb,
                maskt, kqv, exppool, smalls, ps, blk, interleave=None):
    """One attention block: act_in [128, DT, TOK] f32r -> act_out (same shape)."""
    kT = kqv.tile([11, TOK], f32r, tag="kT", name=f"kT{blk}")
    qT = kqv.tile([11, TOK], f32r, tag="qT", name=f"qT{blk}")
    vnat = kqv.tile([128, TOK // 128, HP], f32r, tag="vnat", name=f"vnat{blk}")
    resT = kqv.tile([11, TOK], f32r, tag="resT", name=f"resT{blk}")

    # --- projections k,q: [11, TOK] = w.T @ eT, + bias (bias add on ScalarE) ---
    for ch in range(TOK // SCH):
        sl = slice(ch * SCH, (ch + 1) * SCH)
        for w_sb, b_sb, dest in ((wk, kb, kT), (wq, qb, qT)):
            pst = ps.tile([11, SCH], f32, tag="bank1", bufs=4,
                          name=f"proj{blk}_{ch}")
            for d in range(DT):
                nc.tensor.matmul(pst[:], w_sb[:, d], act_in[:, d, sl],
                                 start=(d == 0), stop=(d == DT - 1))
            nc.scalar.add(dest[:, sl], pst[:], b_sb[:])
    # --- v directly in [tok, HP] layout: eT_tile.T @ wv  (HP=12: f32r matmul
    # needs an even moving free dim, so wv is zero-padded to 12 cols) ---
    for t in range(TOK // 128):
        tsl = slice(t * 128, (t + 1) * 128)
        pst = ps.tile([128, HP], f32, tag="bank1", bufs=4, name=f"vproj{blk}_{t}")
        for d in range(DT):
            nc.tensor.matmul(pst[:], act_in[:, d, tsl], wv[:, d],
                             start=(d == 0), stop=(d == DT - 1))
        nc.vector.tensor_add(vnat[:, t], pst[:], vbb[:])

    # --- attention per batch ---
    for b in range(B):
        base = b * S
        avps = [ps.tile([11, SCH], f32, tag="bank1", bufs=4,
                        name=f"av{blk}_{b}_{c}") for c in range(NSCH_B)]
        for t in range(KT_B):
            c0 = t // 4
            ksl = slice(base + t * 128, base + t * 128 + 128)
            # chunk pairs: (start_chunk, n_chunks) covering c0..3
            pairs = {0: [(0, 2), (2, 2)], 1: [(1, 1), (2, 2)],
                     2: [(2, 2)], 3: [(3, 1)]}[c0]
            exp_t = exppool.tile([128, NSCH_B, SCH], f32r, tag="exp",
                                 name=f"exp{blk}_{b}_{t}")
            sums = smalls.tile([128, 2], f32, tag="sums", name=f"sums{blk}_{b}_{t}")
            for pi, (cs, w) in enumerate(pairs):
                pst = ps.tile([128, 2, SCH], f32, tag="pair", bufs=2,
                              name=f"sc{blk}_{b}_{t}_{pi}")
                for j in range(w):
                    qsl = slice(base + (cs + j) * SCH, base + (cs + j + 1) * SCH)
                    nc.tensor.matmul(pst[:, j], kT[:, ksl], qT[:, qsl],
                                     start=True, stop=True)
                if pi == 0:  # boundary chunk c0 is the first chunk of pair 0
                    nc.vector.tensor_add(pst[:, 0], pst[:, 0], maskt[:, t % 4])
                nc.scalar.activation(exp_t[:, cs:cs + w], pst[:, :w],
                                     mybir.ActivationFunctionType.Exp,
                                     accum_out=sums[:, pi:pi + 1])
            if len(pairs) > 1:
                stot = smalls.tile([128, 1], f32, tag="stot", name=f"st{blk}_{b}_{t}")
                nc.vector.reduce_sum(stot[:], sums[:, :2], axis=mybir.AxisListType.X)
            else:
                stot = sums[:, 0:1]
            rec = smalls.tile([128, 1], f32, tag="rec", name=f"rec{blk}_{b}_{t}")
            nc.vector.reciprocal(rec[:], stot[:])
            vsc = smalls.tile([128, HP], f32r, tag="vsc", name=f"vsc{blk}_{b}_{t}")
            nc.vector.tensor_scalar_mul(vsc[:], vnat[:, b * KT_B + t], rec[:])
            for c in range(c0, NSCH_B):
                nc.tensor.matmul(avps[c][:], vsc[:, :H], exp_t[:, c],
                                 start=(t == 0), stop=(t == 4 * c + 3))
                if t == 4 * c + 3:
                    nc.vector.tensor_copy(
                        resT[:, base + c * SCH:base + (c + 1) * SCH], avps[c][:])
            if b == 1 and interleave is not None:
                interleave(t)

        # --- ff for this batch: act_out[d, tok] = fw.T @ resT + fb ---
        for ch in range(b * NSCH_B, (b + 1) * NSCH_B):
            sl = slice(ch * SCH, (ch + 1) * SCH)
            for d in range(DT):
                pst = ps.tile([128, SCH], f32, tag="bank1", bufs=4,
                              name=f"ff{blk}_{d}_{ch}")
                nc.tensor.matmul(pst[:], fw[:, d * 128:(d + 1) * 128],
                                 resT[:, sl], start=True, stop=True)
                if ch % 2 == 0:
                    nc.vector.tensor_scalar_add(act_out[:, d, sl], pst[:],
                                                fb[:, d:d + 1])
                else:
                    nc.scalar.add(act_out[:, d, sl], pst[:], fb[:, d:d + 1])


def build():
    nc = bacc.Bacc("TRN2", target_bir_lowering=False, debug=False,
                   num_devices=NCORES)
    d_emb = nc.dram_tensor("emb_table", [V, D], f32, kind="ExternalInput").ap()
    d_idx = nc.dram_tensor("idx", [128, TOK // 16], i16, kind="ExternalInput").ap()
    d_pos = nc.dram_tensor("pos_t", [128, DT, S], f32, kind="ExternalInput").ap()
    d_masks = nc.dram_tensor("masks", [128, 4, SCH], f32, kind="ExternalInput").ap()
    d_wkqv = []
    d_bias = []
    for blk in range(2):
        d_wkqv.append([
            nc.dram_tensor(f"w{blk}_k", [128, DT, H], f32, kind="ExternalInput").ap(),
            nc.dram_tensor(f"w{blk}_q", [128, DT, H], f32, kind="ExternalInput").ap(),
            nc.dram_tensor(f"w{blk}_v", [128, DT, HP], f32, kind="ExternalInput").ap()])
        d_wkqv[blk].append(
            nc.dram_tensor(f"w{blk}_f", [H, D], f32, kind="ExternalInput").ap())
        d_bias.append([
            nc.dram_tensor(f"b{blk}_k", [11, 1], f32, kind="ExternalInput").ap(),
            nc.dram_tensor(f"b{blk}_q", [11, 1], f32, kind="ExternalInput").ap(),
            nc.dram_tensor(f"b{blk}_v", [128, HP], f32, kind="ExternalInput").ap()])
        d_bias[blk].append(
            nc.dram_tensor(f"b{blk}_f", [128, DT], f32, kind="ExternalInput").ap())
    d_wout = nc.dram_tensor("w_out", [128, DT, VS], f32, kind="ExternalInput").ap()
    d_bout = nc.dram_tensor("b_out", [128, VS], f32, kind="ExternalInput").ap()
    d_out = nc.dram_tensor("out", [TOK, VS], f32, kind="ExternalOutput").ap()

    with tile.TileContext(nc) as tc:
        import contextlib
        with contextlib.ExitStack() as ctx:
            consts = ctx.enter_context(tc.tile_pool(name="consts", bufs=1))
            acts = ctx.enter_context(tc.tile_pool(name="acts", bufs=1))
            ps = ctx.enter_context(tc.tile_pool(name="ps", bufs=1, space="PSUM"))

            ident = consts.tile([128, 128], f32, tag="ident", name="ident")
            make_identity(nc, ident[:])
            idxt = consts.tile([128, TOK // 16], i16, tag="idx", name="idxt")
            nc.sync.dma_start(idxt[:], d_idx)
            maskt = consts.tile([128, 4, SCH], f32, tag="masks", name="maskt")

            # attention weights/biases (small, resident)
            wb = []
            for blk in range(2):
                wk = consts.tile([128, DT, H], f32r, tag=f"w{blk}k", name=f"w{blk}k")
                wq = consts.tile([128, DT, H], f32r, tag=f"w{blk}q", name=f"w{blk}q")
                wv = consts.tile([128, DT, HP], f32r, tag=f"w{blk}v", name=f"w{blk}v")
                fw = consts.tile([11, D], f32r, tag=f"w{blk}f", name=f"w{blk}f")
                kb = consts.tile([11, 1], f32, tag=f"b{blk}k", name=f"b{blk}k")
                qb = consts.tile([11, 1], f32, tag=f"b{blk}q", name=f"b{blk}q")
                vbb = consts.tile([128, HP], f32, tag=f"b{blk}v", name=f"b{blk}v")
                fb = consts.tile([128, DT], f32, tag=f"b{blk}f", name=f"b{blk}f")
                for t_sb, t_dr in zip((kb, qb, vbb, fb), d_bias[blk]):
                    nc.sync.dma_start(t_sb[:], t_dr)
                wb.append((wk, wq, wv, fw, kb, qb, vbb, fb))

            # vocab-projection weight streaming pool opened early so the first
            # quarters' DMAs can prefetch during attention
            wpool = ctx.enter_context(tc.tile_pool(name="wpool", bufs=2))
            stpool = ctx.enter_context(tc.tile_pool(name="stpool", bufs=4))
            obpool = ctx.enter_context(tc.tile_pool(name="obpool", bufs=2))
            wq_tiles = []

            eT = acts.tile([128, DT, TOK], f32r, tag="act", name="eT")

            # --- embedding gather + transpose + positional encoding ---
            with (tc.tile_pool(name="embed", bufs=2) as epool,
                  tc.tile_pool(name="pospool", bufs=1) as ppool):
                post = ppool.tile([128, DT, S], f32, tag="pos", name="post")
                for g in range(TOK // 512):
                    if g < 4:  # batch-0 pass loads pos just-in-time; batch 1 reuses
                        nc.sync.dma_start(post[:, :, g * 512:(g + 1) * 512],
                                          d_pos[:, :, g * 512:(g + 1) * 512])
                    en = epool.tile([128, 4, D], f32, tag="enat", name=f"en{g}")
                    nc.gpsimd.dma_gather(
                        out_ap=en[:], in_ap=d_emb,
                        idxs_ap=idxt[:, 32 * g:32 * (g + 1)],
                        num_idxs=512, num_idxs_reg=512, elem_size=D)
                    for c4 in range(4):
                        tokbase = g * 512 + c4 * 128
                        sbase = tokbase % S
                        for d in range(DT):
                            pst = ps.tile([128, 128], f32, tag="bank1", bufs=4,
                                          name=f"etr{g}_{c4}_{d}")
                            nc.tensor.transpose(
                                pst[:], en[:, c4, d * 128:(d + 1) * 128], ident[:])
                            nc.vector.tensor_add(
                                eT[:, d, tokbase:tokbase + 128], pst[:],
                                post[:, d, sbase:sbase + 128])

            # attention-weight cast-DMAs emitted after the gathers so they
            # don't delay them on the single SWDGE queue
            for blk in range(2):
                for t_sb, t_dr in zip(wb[blk][:4], d_wkqv[blk]):
                    nc.gpsimd.dma_start(t_sb[:], t_dr)
            # prefetch first W quarter during attention; per-d pieces so the
            # vocab matmuls can start on partial arrival
            nc.sync.dma_start(maskt[:], d_masks)

            def load_wq(q):
                wq_sb = wpool.tile([128, DT, VQ], f32r, tag="W", name=f"W{q}")
                for d in range(DT):
                    nc.gpsimd.dma_start(wq_sb[:, d],
                                        d_wout[:, d, q * VQ:(q + 1) * VQ])
                return wq_sb

            wq_tiles.append(load_wq(0))
            ob_q0 = obpool.tile([128, VQ], f32, tag="ob", name="ob0")
            nc.sync.dma_start(ob_q0[:], d_bout[:, 0:VQ])

            # --- two attention blocks ---
            with (tc.tile_pool(name="kqv", bufs=1) as kqv,
                  tc.tile_pool(name="exppool", bufs=3) as exppool,
                  tc.tile_pool(name="smalls", bufs=6) as smalls):
                res1 = acts.tile([128, DT, TOK], f32r, tag="act", name="res1")
                _attn_block(nc, tc, eT, res1, *wb[0], maskt,
                            kqv, exppool, smalls, ps, 0)
                res2 = acts.tile([128, DT, TOK], f32r, tag="act", name="res2")

                def bigmm_unit(q, t, wq_sb, ob_q):
                    tsl = slice(t * 128, (t + 1) * 128)
                    stage = stpool.tile([128, VQ], f32, tag="stage",
                                        name=f"stg{q}_{t}")
                    for c in range(VQ // VCH):
                        pst = ps.tile([128, VCH], f32, tag="bank1", bufs=4,
                                      name=f"big{q}_{t}_{c}",
                                      padded_shape=[128, 512])
                        csl = slice(c * VCH, (c + 1) * VCH)
                        for d in range(DT):
                            nc.tensor.matmul(pst[:], res2[:, d, tsl],
                                             wq_sb[:, d, csl],
                                             start=(d == 0), stop=(d == DT - 1))
                        nc.vector.tensor_add(stage[:, csl], pst[:], ob_q[:, csl])
                    nc.sync.dma_start(d_out[tsl, q * VQ:(q + 1) * VQ], stage[:])

                # vocab-matmul units for quarter 0 / batch-0 tok-tiles 4..15
                # interleave into block-2's batch-1 loop, using the bank1
                # PSUM slots that the AV accumulators release progressively
                # (av[c] frees at k-tile 4c+3); the score-pair pipeline is
                # untouched, so ACT keeps streaming exp at full rate.
                _attn_block(nc, tc, res1, res2, *wb[1], maskt,
                            kqv, exppool, smalls, ps, 1,
                            interleave=lambda t: (bigmm_unit(
                                0, t, wq_tiles[0], ob_q0) if t >= 4 else None))

            # --- vocab projection: out[tok, v] = res2.T @ W + b ---
            if True:
                for q in range(NQUART):
                    wq_sb = wq_tiles[q] if q < len(wq_tiles) else load_wq(q)
                    if q == 0:
                        ob_q = ob_q0
                    else:
                        ob_q = obpool.tile([128, VQ], f32, tag="ob",
                                           name=f"ob{q}")
                        nc.sync.dma_start(ob_q[:], d_bout[:, q * VQ:(q + 1) * VQ])
                    for t in range(TOK // 128):
                        if q == 0 and 4 <= t < KT_B:
                            continue  # emitted interleaved with attention
                        bigmm_unit(q, t, wq_sb, ob_q)

    nc.compile()
    return nc


def _prep_inputs(inputs):
    x = np.asarray(inputs["x"]).reshape(-1).astype(np.int64)
    assert x.size == TOK
    idx16 = x.astype(np.int16)
    idx_w = np.tile(np.ascontiguousarray(idx16.reshape(TOK // 16, 16).T), (8, 1))

    pos_t = _rearr_w(np.ascontiguousarray(_pos_matrix_np().T))  # [128, DT, S]
    masks = np.ascontiguousarray(_masks_np().transpose(1, 0, 2))  # [128, 4, SCH]

    common = {
        "emb_table": np.ascontiguousarray(np.asarray(inputs["emb_table"], dtype=np.float32)),
        "idx": np.ascontiguousarray(idx_w),
        "pos_t": np.ascontiguousarray(pos_t),
        "masks": masks,
    }
    for blk, pre in ((0, "1"), (1, "2")):
        for n, key in (("k", f"k{pre}_w"), ("q", f"q{pre}_w")):
            common[f"w{blk}_{n}"] = _rearr_w(np.asarray(inputs[key], dtype=np.float32))
        wv_pad = np.zeros((D, HP), dtype=np.float32)
        wv_pad[:, :H] = np.asarray(inputs[f"v{pre}_w"], dtype=np.float32)
        common[f"w{blk}_v"] = _rearr_w(wv_pad)
        common[f"w{blk}_f"] = np.ascontiguousarray(
            np.asarray(inputs[f"ff{pre}_w"], dtype=np.float32))
        for n, key in (("k", f"k{pre}_b"), ("q", f"q{pre}_b")):
            common[f"b{blk}_{n}"] = np.ascontiguousarray(
                np.asarray(inputs[key], dtype=np.float32).reshape(11, 1))
        vb_pad = np.zeros(HP, dtype=np.float32)
        vb_pad[:H] = np.asarray(inputs[f"v{pre}_b"], dtype=np.float32)
        common[f"b{blk}_v"] = np.ascontiguousarray(
            np.broadcast_to(vb_pad, (128, HP)))
        common[f"b{blk}_f"] = np.ascontiguousarray(
            np.asarray(inputs[f"ff{pre}_b"], dtype=np.float32).reshape(DT, 128).T)

    out_w = np.asarray(inputs["out_w"], dtype=np.float32)
    out_b = np.asarray(inputs["out_b"], dtype=np.float32)
    in_maps = []
    for c in range(NCORES):
        m = dict(common)
        m["w_out"] = _rearr_w(np.ascontiguousarray(out_w[:, c * VS:(c + 1) * VS]))
        m["b_out"] = np.ascontiguousarray(
            np.broadcast_to(out_b[c * VS:(c + 1) * VS], (128, VS)))
        in_maps.append(m)
    return in_maps


_last_results = None


def kernel(**inputs) -> np.ndarray:
    global _last_results
    if "nc" not in _cached:
        _cached["nc"] = build()
    nc = _cached["nc"]
    in_maps = _prep_inputs(inputs)
    res = run_bass_kernel_spmd(nc, in_maps, core_ids=list(range(NCORES)))
    _last_results = res
    out = np.concatenate([res.results[c]["out"] for c in range(NCORES)], axis=1)
    return out.reshape(B, S, V)
